# revision 39
# baseline (speedup 1.0000x reference)
"""Trainium2 Bass kernel for MinimalLightningAttention2.

Strategy (8 NeuronCores, SPMD, no collectives):
  core c -> batch b = c // 4, head group g = c % 4 (heads 4g..4g+3).
  Each core computes, fully fused on-chip:
    qkv projection (its 4 heads' columns of Wqkv)
    chunked lightning-attention scan (L=128 chunks, per-head decay state S)
    row-parallel partial of the output projection (its 4 heads' rows of Wout)
  Host sums the 4 partial outputs per batch and adds bout.

Layouts on device (per core):
  xT   [c, it, kt, n']  host-pre-transposed/packed bf16 x, span-major so each
                        span's DMA is one 16KB-contiguous run per partition
  q,k  [d,   n]  (lhsT = Wq/Wk tile, rhs = xT)
  v    [n, h*d]  (lhsT = xT tile,    rhs = Wv)
  attn output oT [e, i] per head -> directly the lhsT of the Wout matmul.
All matmuls in bf16 (PSUM accumulation fp32); decay masks applied in fp32
during PSUM eviction; decay state S kept in fp32 with a bf16 shadow.

Perf notes (measured on trn2 via NTFF traces, ~504us, was 523us):
  - tensor engine active ~487us of ~504us wall (pure-stream floor ~476us);
    HAM clock gate stays at K=8/8 (2.4GHz) for the entire run.
  - the startup DMA ramp is ~75->270->400GB/s over the first ~26us and is
    per-TILE bound early (~1.3us/tile regardless of size), identical for 1
    or 2 HWDGE rings, so: one ring, big quarter tiles, in need-order.
  - 36 dependency-free warm-up matmuls run first (overlapping the initial
    DMA wait) so HAM is warm before the first real matmul and the first
    stripe boundary lands past the jitter-exposed part of the ramp (with
    fewer warmups, ~half the runs hit a ~3.5us re-throttle at ~16-21us).
  - span-0 runs Q AND K kt-striped in 8 parallel PSUM banks (K borrows the
    idle attention/output banks) so each arriving x/wq/wk quarter unlocks
    32 matmuls - the PE never idles long enough to re-throttle.
  - q_raw eviction on the scalar engine, q_dec on vector (parallel PSUM
    reads); S decay-multiply runs early and the bf16 shadow of S is
    written before the f32 state so the inter-chunk recurrence is short.
  - output partials are bf16 on the sync HWDGE ring; the last chunk
    streams out piece-wise on alternating rings to shorten the tail flush.
  - remaining fixed costs: ~4.5us pre-first-data, ~6.1us of framework
    per-semaphore resets at the end, ~1.4us NEFF barriers.
"""

import math

import numpy as np
import ml_dtypes

B, N, C = 2, 4096, 2048
H_TOT = 16
HD = 128          # head dim
H = 4             # heads per core
L = 128           # attention chunk length
NCH = N // L      # 32 chunks
KT = C // 128     # 16 contraction tiles for the projections
NSPAN = 512       # tokens per outer iteration
NIT = N // NSPAN  # 8 outer iterations
P = 128
XE = KT // 8      # kt tiles per span-0 eighth

BF16 = ml_dtypes.bfloat16

_CACHE = {}


def _build():
    """Build + compile the SPMD Bass program (same program on all 8 cores)."""
    from contextlib import ExitStack

    import concourse.bass as bass
    import concourse.tile as tile
    from concourse import bacc, mybir

    DT = mybir.dt.bfloat16
    F32 = mybir.dt.float32

    nc = bacc.Bacc(
        "TRN2",
        target_bir_lowering=False,
        debug=False,
        enable_asserts=False,
        num_devices=8,
    )

    # host-packed transpose of x: xtp[c, it, kt, n'] = x[it*512 + n', kt*128 + c]
    xd = nc.dram_tensor("x", [P, NIT, KT, NSPAN], DT, kind="ExternalInput").ap()
    # host-packed: [c, kt*512 + col] (col = head*128 + d), fully contiguous rows
    wqd = nc.dram_tensor("wq", [P, KT * 512], DT, kind="ExternalInput").ap()
    wkd = nc.dram_tensor("wk", [P, KT * 512], DT, kind="ExternalInput").ap()
    wvd = nc.dram_tensor("wv", [P, KT * 512], DT, kind="ExternalInput").ap()
    # host-packed: [d, h*2048 + outc]
    wod = nc.dram_tensor("wo", [P, H * C], DT, kind="ExternalInput").ap()
    masktd = nc.dram_tensor("maskt", [P, H * L], F32, kind="ExternalInput").ap()
    qdecd = nc.dram_tensor("qdec", [P, H * NSPAN], F32, kind="ExternalInput").ap()
    kdecvd = nc.dram_tensor("kdecv", [P, H * HD], F32, kind="ExternalInput").ap()
    bdfd = nc.dram_tensor("bdf", [P, H * HD], F32, kind="ExternalInput").ap()
    bqkd = nc.dram_tensor("bqk", [P, 2 * H], F32, kind="ExternalInput").ap()
    bvfd = nc.dram_tensor("bvf", [P, H * HD], F32, kind="ExternalInput").ap()
    outd = nc.dram_tensor("out", [N, C], DT, kind="ExternalOutput").ap()

    mult = mybir.AluOpType.mult
    add = mybir.AluOpType.add

    with tile.TileContext(nc) as tc:
        with ExitStack() as ctx:
            const = ctx.enter_context(tc.tile_pool(name="const", bufs=1))
            xt0_pool = ctx.enter_context(tc.tile_pool(name="xt0", bufs=1))
            xt_pool = ctx.enter_context(tc.tile_pool(name="xt", bufs=2))
            qk_pool = ctx.enter_context(tc.tile_pool(name="qk", bufs=2))
            sc_pool = ctx.enter_context(tc.tile_pool(name="sc", bufs=3))
            ob_pool = ctx.enter_context(tc.tile_pool(name="ob", bufs=3))
            outb_pool = ctx.enter_context(tc.tile_pool(name="outb", bufs=2))
            qkv_ps = ctx.enter_context(tc.tile_pool(name="qkvps", bufs=4, space="PSUM"))
            attn_ps = ctx.enter_context(tc.tile_pool(name="attnps", bufs=1, space="PSUM"))
            out_ps = ctx.enter_context(tc.tile_pool(name="outps", bufs=2, space="PSUM"))

            # ---- PE warm-up: dependency-free matmuls so the HAM clock gate
            # reaches K=8/8 (~3.4us sustained busy) while the first DMAs are
            # still in flight. Results land in a qkv PSUM slot that real
            # projections overwrite (start=True clears the bank).
            warm_sb = const.tile([P, 512], DT, name="warm_sb")
            nc.vector.memset(warm_sb[:], 0.0)
            warm_ps = qkv_ps.tile([P, 512], F32, tag="qkvps", name="warm_ps")
            for _ in range(36):
                nc.tensor.matmul(
                    warm_ps[:], lhsT=warm_sb[:, 0:P], rhs=warm_sb[:],
                    start=True, stop=True,
                )

            # ---- constants / weights resident in SBUF ----
            # All big startup loads on the ONE sync ring, in need-order: each
            # DGE ring is a FIFO and rings fair-share HBM at packet
            # granularity. First x-span + wq go in interleaved 0.25MB eighths
            # (2KB-contiguous per partition) so the PE can start after 0.5MB.
            # Small decay/bias constants go on the gpsimd (SWDGE) ring.
            # The early DMA phase is descriptor-supply-bound per ring
            # (~1.3us/tile regardless of size), so: big quarter tiles, and
            # the two HWDGE rings (sync=SP, scalar=Activation) in parallel —
            # x on the sync ring, all weights on the scalar ring.
            piece_k0 = [0, 6, 10, 13, 16]
            xt0_p = []
            wq_p = []
            wk_h = []
            for e in range(len(piece_k0) - 1):
                k0, k1 = piece_k0[e], piece_k0[e + 1]
                xe = xt0_pool.tile([P, k1 - k0, NSPAN], DT, tag=f"xte{e}", name=f"xt0e{e}")
                nc.sync.dma_start(xe[:], xd[:, 0, k0:k1, :])
                xt0_p.append(xe)
                wt = const.tile([P, (k1 - k0) * 512], DT, tag=f"wqe{e}", name=f"wqe{e}")
                nc.sync.dma_start(wt[:], wqd[:, k0 * 512:k1 * 512])
                wq_p.append(wt)
                t = const.tile([P, (k1 - k0) * 512], DT, tag=f"wkh{e}", name=f"wkh{e}")
                nc.sync.dma_start(t[:], wkd[:, k0 * 512:k1 * 512])
                wk_h.append(t)

            def piece_of(kt):
                for e in range(len(piece_k0) - 1):
                    if piece_k0[e] <= kt < piece_k0[e + 1]:
                        return e, kt - piece_k0[e]
                raise AssertionError
            wv_sb = const.tile([P, KT * 512], DT)
            nc.sync.dma_start(wv_sb[:], wvd[:])
            wo_sb = const.tile([P, H * C], DT)
            nc.sync.dma_start(wo_sb[:], wod[:])
            qdec_sb = const.tile([P, H * NSPAN], F32)
            nc.gpsimd.dma_start(qdec_sb[:], qdecd[:])
            bqk_sb = const.tile([P, 2 * H], F32)
            nc.gpsimd.dma_start(bqk_sb[:], bqkd[:])
            kdecv_sb = const.tile([P, H * HD], F32)
            nc.gpsimd.dma_start(kdecv_sb[:], kdecvd[:])
            bdf_sb = const.tile([P, H * HD], F32)
            nc.gpsimd.dma_start(bdf_sb[:], bdfd[:])
            bvf_sb = const.tile([P, H * HD], F32)
            nc.gpsimd.dma_start(bvf_sb[:], bvfd[:])
            maskt_sb = const.tile([P, H * L], F32)
            nc.gpsimd.dma_start(maskt_sb[:], masktd[:])
            ident = const.tile([P, P], DT)
            from concourse.masks import make_identity
            make_identity(nc, ident)

            # per-head decay state S [d, e], 4 heads side by side, fp32
            S_sb = const.tile([P, H * HD], F32)
            nc.vector.memset(S_sb[:], 0.0)
            S_bf = const.tile([P, H * HD], DT)
            nc.vector.memset(S_bf[:], 0.0)

            xt_tiles = [None]
            for it in range(NIT):
                n0 = it * NSPAN
                # prefetch next span's xT (host-packed, one contiguous tile)
                if it + 1 < NIT:
                    xs = xt_pool.tile([P, KT, NSPAN], DT, tag="xsp", name=f"xsp{it + 1}")
                    nc.sync.dma_start(xs[:], xd[:, it + 1, :, :])
                    xt_tiles.append(xs)

                def wq_sl(kt, h):
                    e, o = piece_of(kt)
                    return wq_p[e][:, o * 512 + h * HD: o * 512 + (h + 1) * HD]

                if it == 0:
                    def xts(kt):
                        e, o = piece_of(kt)
                        return xt0_p[e][:, o, :]
                else:
                    xtab = xt_tiles[it]

                    def xts(kt, xtab=xtab):
                        return xtab[:, kt, :]

                # ---- qkv projection for the span ----
                q_raw = qk_pool.tile([P, H * NSPAN], DT, tag="q_raw")
                q_dec = qk_pool.tile([P, H * NSPAN], DT, tag="q_dec")
                k_sb = qk_pool.tile([P, H * NSPAN], DT, tag="k_sb")
                v_sb = qk_pool.tile([P, H * NSPAN], DT, tag="v_sb")
                vdec = qk_pool.tile([P, H * NSPAN], DT, tag="vdec")

                def wk_sl(kt, h):
                    e, o = piece_of(kt)
                    return wk_h[e][:, o * 512 + h * HD: o * 512 + (h + 1) * HD]

                def evict_q(ps, h):
                    # q_raw = psum + bq (scalar engine); q_dec = (psum + bq) *
                    # qdec (vector engine) — independent reads of the PSUM
                    # bank so the two evictions run in parallel.
                    nc.scalar.activation(
                        q_raw[:, h * NSPAN:(h + 1) * NSPAN], ps[:],
                        mybir.ActivationFunctionType.Identity, bias=bqk_sb[:, 2 * h:2 * h + 1],
                    )
                    nc.vector.scalar_tensor_tensor(
                        q_dec[:, h * NSPAN:(h + 1) * NSPAN], ps[:], bqk_sb[:, 2 * h:2 * h + 1],
                        qdec_sb[:, h * NSPAN:(h + 1) * NSPAN], op0=add, op1=mult,
                    )

                def evict_k(ps, h):
                    nc.scalar.activation(
                        k_sb[:, h * NSPAN:(h + 1) * NSPAN], ps[:],
                        mybir.ActivationFunctionType.Identity, bias=bqk_sb[:, 2 * h + 1:2 * h + 2],
                    )

                if it == 0:
                    # Startup is DMA-gated: run Q AND K kt-striped in 8
                    # parallel PSUM banks (Q in the qkv pool, K borrowing the
                    # idle attention/output banks) so every arriving
                    # x/wq/wk quarter-stripe unlocks 32 matmuls — the PE
                    # stays saturated (and HAM stays warm) through the ramp.
                    psq = [qkv_ps.tile([P, NSPAN], F32, tag="qkvps", name=f"q0ps{h}") for h in range(H)]
                    psk = [
                        attn_ps.tile([P, NSPAN], F32, tag="sc", name="k0ps0"),
                        attn_ps.tile([P, NSPAN], F32, tag="o", name="k0ps1"),
                        out_ps.tile([P, NSPAN], F32, tag="outps", name="k0ps2"),
                        out_ps.tile([P, NSPAN], F32, tag="outps", name="k0ps3"),
                    ]
                    for e in range(len(piece_k0) - 1):
                        k0, k1 = piece_k0[e], piece_k0[e + 1]
                        for kt in range(k0, k1):
                            for h in range(H):
                                nc.tensor.matmul(
                                    psq[h][:], lhsT=wq_sl(kt, h), rhs=xts(kt),
                                    start=(kt == 0), stop=(kt == KT - 1),
                                )
                        for kt in range(k0, k1):
                            for h in range(H):
                                nc.tensor.matmul(
                                    psk[h][:], lhsT=wk_sl(kt, h), rhs=xts(kt),
                                    start=(kt == 0), stop=(kt == KT - 1),
                                )

                    for h in range(H):
                        evict_q(psq[h], h)
                    for h in range(H):
                        evict_k(psk[h], h)
                else:
                    for h in range(H):
                        ps = qkv_ps.tile([P, NSPAN], F32, tag="qkvps")
                        for kt in range(KT):
                            nc.tensor.matmul(
                                ps[:], lhsT=wq_sl(kt, h), rhs=xts(kt),
                                start=(kt == 0), stop=(kt == KT - 1),
                            )
                        evict_q(ps, h)
                        ps = qkv_ps.tile([P, NSPAN], F32, tag="qkvps")
                        for kt in range(KT):
                            nc.tensor.matmul(
                                ps[:], lhsT=wk_sl(kt, h), rhs=xts(kt),
                                start=(kt == 0), stop=(kt == KT - 1),
                            )
                        evict_k(ps, h)

                for ns in range(4):
                    ps = qkv_ps.tile([P, NSPAN], F32, tag="qkvps")
                    for kt in range(KT):
                        nc.tensor.matmul(
                            ps[:],
                            lhsT=xts(kt)[:, ns * P:(ns + 1) * P],
                            rhs=wv_sb[:, kt * 512:(kt + 1) * 512],
                            start=(kt == 0), stop=(kt == KT - 1),
                        )
                    nc.vector.tensor_tensor(v_sb[:, ns * 512:(ns + 1) * 512], ps[:], bvf_sb[:], op=add)
                    nc.vector.tensor_tensor(vdec[:, ns * 512:(ns + 1) * 512], v_sb[:, ns * 512:(ns + 1) * 512], kdecv_sb[:], op=mult)


                # ---- attention + output projection, chunk by chunk ----
                # In the last span the qkv PSUM banks are idle; borrow them so
                # the attention chunks can overlap instead of serializing on
                # the single attn bank pair.
                last = it == NIT - 1
                for p in range(4):
                    ap_pool = qkv_ps if last else attn_ps
                    ap_tag = "qkvps" if last else "sc"
                    ao_tag = "qkvps" if last else "o"
                    # scoresT for all 4 heads into one psum bank
                    sc_ps = ap_pool.tile([P, 512], F32, tag=ap_tag, name=f"sc_ps{p}")
                    for h in range(H):
                        nc.tensor.matmul(
                            sc_ps[:, h * L:(h + 1) * L],
                            lhsT=k_sb[:, h * NSPAN + p * L: h * NSPAN + (p + 1) * L],
                            rhs=q_raw[:, h * NSPAN + p * L: h * NSPAN + (p + 1) * L],
                            start=True, stop=True,
                        )
                    scT = sc_pool.tile([P, 512], DT, tag="scT")
                    nc.vector.tensor_tensor(scT[:], sc_ps[:], maskt_sb[:], op=mult)

                    # decay the state early (S_sb is not read by the PE; only
                    # its bf16 shadow is) so the post-su update is 2 ops.
                    nc.vector.tensor_tensor(S_sb[:], S_sb[:], bdf_sb[:], op=mult)

                    # kT (transpose k chunk) for all 4 heads
                    kt_ps = ap_pool.tile([P, 512], DT, tag=ap_tag, name=f"kt_ps{p}")
                    for h in range(H):
                        nc.tensor.transpose(
                            kt_ps[:, h * HD:(h + 1) * HD],
                            k_sb[:, h * NSPAN + p * L: h * NSPAN + (p + 1) * L],
                            ident[:],
                        )
                    kT = sc_pool.tile([P, 512], DT, tag="kT")
                    nc.scalar.copy(kT[:], kt_ps[:])

                    # o = v^T @ scoresT + S^T @ qdec   [e, i] per head
                    o_ps = ap_pool.tile([P, 512], F32, tag=ao_tag, name=f"o_ps{p}")
                    for h in range(H):
                        nc.tensor.matmul(
                            o_ps[:, h * L:(h + 1) * L],
                            lhsT=v_sb[:, p * 512 + h * HD: p * 512 + (h + 1) * HD],
                            rhs=scT[:, h * L:(h + 1) * L],
                            start=True, stop=False,
                        )
                        nc.tensor.matmul(
                            o_ps[:, h * L:(h + 1) * L],
                            lhsT=S_bf[:, h * HD:(h + 1) * HD],
                            rhs=q_dec[:, h * NSPAN + p * L: h * NSPAN + (p + 1) * L],
                            start=False, stop=True,
                        )
                    ob = ob_pool.tile([P, 512], DT, tag="ob")
                    nc.vector.tensor_copy(ob[:], o_ps[:])

                    # S <- S (pre-decayed above) + kT^T @ vdec
                    su_ps = ap_pool.tile([P, 512], F32, tag=ao_tag, name=f"su_ps{p}")
                    for h in range(H):
                        nc.tensor.matmul(
                            su_ps[:, h * HD:(h + 1) * HD],
                            lhsT=kT[:, h * HD:(h + 1) * HD],
                            rhs=vdec[:, p * 512 + h * HD: p * 512 + (h + 1) * HD],
                            start=True, stop=True,
                        )
                    # bf16 shadow first — the next chunk's o-inter matmul
                    # only needs S_bf; the f32 state update follows off the
                    # critical path.
                    nc.vector.tensor_tensor(S_bf[:], S_sb[:], su_ps[:], op=add)
                    nc.vector.tensor_tensor(S_sb[:], S_sb[:], su_ps[:], op=add)

                    # output projection for this chunk's 128 tokens (bf16 partials)
                    outb = outb_pool.tile([P, C], DT, tag="outb")
                    for ct in range(4):
                        ops = out_ps.tile([P, 512], F32, tag="outps")
                        for h in range(H):
                            nc.tensor.matmul(
                                ops[:],
                                lhsT=ob[:, h * L:(h + 1) * L],
                                rhs=wo_sb[:, h * C + ct * 512: h * C + (ct + 1) * 512],
                                start=(h == 0), stop=(h == H - 1),
                            )
                        if last and p == 3 and ct == 3:
                            # Final piece: evict in halves on both engines
                            # and flush each half on its own HWDGE ring so
                            # the kernel tail is as short as possible.
                            c0 = ct * 512
                            nc.vector.tensor_copy(outb[:, c0:c0 + 256], ops[:, 0:256])
                            nc.scalar.copy(outb[:, c0 + 256:c0 + 512], ops[:, 256:512])
                            nc.sync.dma_start(
                                outd[n0 + p * L: n0 + (p + 1) * L, c0:c0 + 256],
                                outb[:, c0:c0 + 256],
                            )
                            nc.scalar.dma_start(
                                outd[n0 + p * L: n0 + (p + 1) * L, c0 + 256:c0 + 512],
                                outb[:, c0 + 256:c0 + 512],
                            )
                            continue
                        if ct % 2 == 0:
                            nc.vector.tensor_copy(outb[:, ct * 512:(ct + 1) * 512], ops[:])
                        else:
                            nc.scalar.copy(outb[:, ct * 512:(ct + 1) * 512], ops[:])
                        # The very last chunk streams out piece-wise on
                        # alternating HWDGE rings so the final flush overlaps
                        # the out-projection instead of trailing it.
                        if last and p == 3:
                            ring = nc.sync if ct % 2 == 0 else nc.scalar
                            ring.dma_start(
                                outd[n0 + p * L: n0 + (p + 1) * L, ct * 512:(ct + 1) * 512],
                                outb[:, ct * 512:(ct + 1) * 512],
                            )
                    # outputs ride the fast HWDGE sync ring (idle mid-run);
                    # the slow SWDGE ring only carries the small const loads.
                    if last and p == 3:
                        pass
                    elif last and p == 2:
                        nc.scalar.dma_start(outd[n0 + p * L: n0 + (p + 1) * L, :], outb[:])
                    else:
                        nc.sync.dma_start(outd[n0 + p * L: n0 + (p + 1) * L, :], outb[:])

    nc.compile()
    return nc


def _host_inputs(x, Wqkv, bqkv, Wout, bout, slopes):
    """Per-core input maps (numpy, host-side sharding + packing)."""
    in_maps = []
    # packed transpose of x, shared by the 4 cores of each batch:
    # xtp[c, it, kt, n'] = x[b, it*512 + n', kt*128 + c]
    _xtp_cache = [
        np.ascontiguousarray(
            x[b].astype(BF16).reshape(NIT, NSPAN, KT, P).transpose(3, 0, 2, 1)
        )
        for b in range(B)
    ]
    i = np.arange(L, dtype=np.float64)
    for core in range(8):
        b, g = core // 4, core % 4
        h0 = 4 * g
        hsel = slice(h0 * HD, (h0 + H) * HD)

        xb = _xtp_cache[b]

        def pack_w(Wslice):
            # (C, 512) -> [c_in_tile(128), kt*512 + col]
            return np.ascontiguousarray(
                Wslice.astype(BF16).reshape(KT, P, H * HD).transpose(1, 0, 2).reshape(P, KT * 512)
            )

        wq = pack_w(Wqkv[:, 0 * C:1 * C][:, hsel])
        wk = pack_w(Wqkv[:, 1 * C:2 * C][:, hsel])
        wv = pack_w(Wqkv[:, 2 * C:3 * C][:, hsel])
        # Wout rows for these heads: [d(128), h*2048 + outc]
        wo = np.ascontiguousarray(
            Wout[hsel, :].astype(BF16).reshape(H, HD, C).transpose(1, 0, 2).reshape(P, H * C)
        )

        s = slopes[h0:h0 + H].astype(np.float64)  # (4,)
        diffT = (i[None, :] - i[:, None])          # [j, i] = i - j
        maskt = np.concatenate(
            [np.where(diffT >= 0, np.exp(-s[h] * diffT), 0.0) for h in range(H)],
            axis=1,
        ).astype(np.float32)                       # [128, 4*128]
        qdec_l = [np.exp(-s[h] * i) for h in range(H)]        # each (L,)
        qdec = np.concatenate(
            [np.broadcast_to(np.tile(qdec_l[h], NSPAN // L)[None, :], (P, NSPAN)) for h in range(H)],
            axis=1,
        ).astype(np.float32)                       # [128, 4*512]
        kdecv = np.concatenate(
            [np.broadcast_to(np.exp(-s[h] * (L - i))[:, None], (P, HD)) for h in range(H)],
            axis=1,
        ).astype(np.float32)                       # [128, 4*128]
        bdf = np.concatenate(
            [np.full((P, HD), math.exp(-s[h] * L)) for h in range(H)], axis=1
        ).astype(np.float32)
        # per-head, per-partition(d) q/k biases: columns [bq_h0, bk_h0, bq_h1, ...]
        bq_heads = bqkv[0 * C:1 * C][hsel].reshape(H, HD)
        bk_heads = bqkv[1 * C:2 * C][hsel].reshape(H, HD)
        bqk = np.zeros((P, 2 * H), dtype=np.float32)
        for h in range(H):
            bqk[:, 2 * h] = bq_heads[h]
            bqk[:, 2 * h + 1] = bk_heads[h]
        bvf = np.broadcast_to(bqkv[2 * C:3 * C][hsel][None, :], (P, H * HD)).astype(np.float32)

        in_maps.append({
            "x": xb, "wq": wq, "wk": wk, "wv": wv, "wo": wo,
            "maskt": maskt, "qdec": qdec, "kdecv": kdecv, "bdf": bdf,
            "bqk": bqk, "bvf": np.ascontiguousarray(bvf),
        })
    return in_maps


def kernel(x, Wqkv, bqkv, Wout, bout, slopes, _want_trace=False):
    from concourse import bass_utils

    x = np.asarray(x, dtype=np.float32)
    Wqkv = np.asarray(Wqkv, dtype=np.float32)
    bqkv = np.asarray(bqkv, dtype=np.float32)
    Wout = np.asarray(Wout, dtype=np.float32)
    bout = np.asarray(bout, dtype=np.float32)
    slopes = np.asarray(slopes, dtype=np.float32)

    if "nc" not in _CACHE:
        _CACHE["nc"] = _build()
    nc = _CACHE["nc"]

    in_maps = _host_inputs(x, Wqkv, bqkv, Wout, bout, slopes)
    res = bass_utils.run_bass_kernel_spmd(
        nc, in_maps, core_ids=list(range(8)), trace=_want_trace,
    )
    out = np.zeros((B, N, C), dtype=np.float32)
    for core in range(8):
        out[core // 4] += res.results[core]["out"].astype(np.float32)
    out += bout[None, None, :]
    if _want_trace:
        _CACHE["last_result"] = res
    return out


# revision 41
# speedup vs baseline: 1.0007x; 1.0007x over previous
"""Trainium2 Bass kernel for MinimalLightningAttention2.

Strategy (8 NeuronCores, SPMD, no collectives):
  core c -> batch b = c // 4, head group g = c % 4 (heads 4g..4g+3).
  Each core computes, fully fused on-chip:
    qkv projection (its 4 heads' columns of Wqkv)
    chunked lightning-attention scan (L=128 chunks, per-head decay state S)
    row-parallel partial of the output projection (its 4 heads' rows of Wout)
  Host sums the 4 partial outputs per batch and adds bout.

Layouts on device (per core):
  xT   [c, it, kt, n']  host-pre-transposed/packed bf16 x, span-major so each
                        span's DMA is one 16KB-contiguous run per partition
  q,k  [d,   n]  (lhsT = Wq/Wk tile, rhs = xT)
  v    [n, h*d]  (lhsT = xT tile,    rhs = Wv)
  attn output oT [e, i] per head -> directly the lhsT of the Wout matmul.
All matmuls in bf16 (PSUM accumulation fp32); decay masks applied in fp32
during PSUM eviction; decay state S kept in fp32 with a bf16 shadow.

Perf notes (measured on trn2 via NTFF traces, ~504us, was 523us):
  - tensor engine active ~487us of ~504us wall (pure-stream floor ~476us);
    HAM clock gate stays at K=8/8 (2.4GHz) for the entire run.
  - the startup DMA ramp is ~75->270->400GB/s over the first ~26us and is
    per-TILE bound early (~1.3us/tile regardless of size), identical for 1
    or 2 HWDGE rings, so: one ring, big quarter tiles, in need-order.
  - 36 dependency-free warm-up matmuls run first (overlapping the initial
    DMA wait) so HAM is warm before the first real matmul and the first
    stripe boundary lands past the jitter-exposed part of the ramp (with
    fewer warmups, ~half the runs hit a ~3.5us re-throttle at ~16-21us).
  - span-0 runs Q AND K kt-striped in 8 parallel PSUM banks (K borrows the
    idle attention/output banks) so each arriving x/wq/wk quarter unlocks
    32 matmuls - the PE never idles long enough to re-throttle.
  - q_raw eviction on the scalar engine, q_dec on vector (parallel PSUM
    reads); S decay-multiply runs early and the bf16 shadow of S is
    written before the f32 state so the inter-chunk recurrence is short.
  - output partials are bf16 on the sync HWDGE ring; the last chunk
    streams out piece-wise on alternating rings to shorten the tail flush.
  - remaining fixed costs: ~4.5us pre-first-data, ~6.1us of framework
    per-semaphore resets at the end, ~1.4us NEFF barriers.
"""

import math

import numpy as np
import ml_dtypes

B, N, C = 2, 4096, 2048
H_TOT = 16
HD = 128          # head dim
H = 4             # heads per core
L = 128           # attention chunk length
NCH = N // L      # 32 chunks
KT = C // 128     # 16 contraction tiles for the projections
NSPAN = 512       # tokens per outer iteration
NIT = N // NSPAN  # 8 outer iterations
P = 128
XE = KT // 8      # kt tiles per span-0 eighth

BF16 = ml_dtypes.bfloat16

_CACHE = {}


def _build():
    """Build + compile the SPMD Bass program (same program on all 8 cores)."""
    from contextlib import ExitStack

    import concourse.bass as bass
    import concourse.tile as tile
    from concourse import bacc, mybir

    DT = mybir.dt.bfloat16
    F32 = mybir.dt.float32

    nc = bacc.Bacc(
        "TRN2",
        target_bir_lowering=False,
        debug=False,
        enable_asserts=False,
        num_devices=8,
    )

    # host-packed transpose of x: xtp[c, it, kt, n'] = x[it*512 + n', kt*128 + c]
    xd = nc.dram_tensor("x", [P, NIT, KT, NSPAN], DT, kind="ExternalInput").ap()
    # host-packed: [c, kt*512 + col] (col = head*128 + d), fully contiguous rows
    wqd = nc.dram_tensor("wq", [P, KT * 512], DT, kind="ExternalInput").ap()
    wkd = nc.dram_tensor("wk", [P, KT * 512], DT, kind="ExternalInput").ap()
    wvd = nc.dram_tensor("wv", [P, KT * 512], DT, kind="ExternalInput").ap()
    # host-packed: [d, h*2048 + outc]
    wod = nc.dram_tensor("wo", [P, H * C], DT, kind="ExternalInput").ap()
    masktd = nc.dram_tensor("maskt", [P, H * L], F32, kind="ExternalInput").ap()
    qdecd = nc.dram_tensor("qdec", [P, H * NSPAN], F32, kind="ExternalInput").ap()
    kdecvd = nc.dram_tensor("kdecv", [P, H * HD], F32, kind="ExternalInput").ap()
    bdfd = nc.dram_tensor("bdf", [P, H * HD], F32, kind="ExternalInput").ap()
    bqkd = nc.dram_tensor("bqk", [P, 2 * H], F32, kind="ExternalInput").ap()
    bvfd = nc.dram_tensor("bvf", [P, H * HD], F32, kind="ExternalInput").ap()
    outd = nc.dram_tensor("out", [N, C], DT, kind="ExternalOutput").ap()

    mult = mybir.AluOpType.mult
    add = mybir.AluOpType.add

    with tile.TileContext(nc) as tc:
        with ExitStack() as ctx:
            const = ctx.enter_context(tc.tile_pool(name="const", bufs=1))
            xt0_pool = ctx.enter_context(tc.tile_pool(name="xt0", bufs=1))
            xt_pool = ctx.enter_context(tc.tile_pool(name="xt", bufs=2))
            qk_pool = ctx.enter_context(tc.tile_pool(name="qk", bufs=2))
            sc_pool = ctx.enter_context(tc.tile_pool(name="sc", bufs=3))
            ob_pool = ctx.enter_context(tc.tile_pool(name="ob", bufs=3))
            outb_pool = ctx.enter_context(tc.tile_pool(name="outb", bufs=2))
            qkv_ps = ctx.enter_context(tc.tile_pool(name="qkvps", bufs=4, space="PSUM"))
            attn_ps = ctx.enter_context(tc.tile_pool(name="attnps", bufs=1, space="PSUM"))
            out_ps = ctx.enter_context(tc.tile_pool(name="outps", bufs=2, space="PSUM"))

            # ---- PE warm-up: dependency-free matmuls so the HAM clock gate
            # reaches K=8/8 (~3.4us sustained busy) while the first DMAs are
            # still in flight. Results land in a qkv PSUM slot that real
            # projections overwrite (start=True clears the bank).
            warm_sb = const.tile([P, 512], DT, name="warm_sb")
            nc.vector.memset(warm_sb[:], 0.0)
            warm_ps = qkv_ps.tile([P, 512], F32, tag="qkvps", name="warm_ps")
            for _ in range(36):
                nc.tensor.matmul(
                    warm_ps[:], lhsT=warm_sb[:, 0:P], rhs=warm_sb[:],
                    start=True, stop=True,
                )

            # ---- constants / weights resident in SBUF ----
            # All big startup loads on the ONE sync ring, in need-order: each
            # DGE ring is a FIFO and rings fair-share HBM at packet
            # granularity. First x-span + wq go in interleaved 0.25MB eighths
            # (2KB-contiguous per partition) so the PE can start after 0.5MB.
            # Small decay/bias constants go on the gpsimd (SWDGE) ring.
            # The early DMA phase is descriptor-supply-bound per ring
            # (~1.3us/tile regardless of size), so: big quarter tiles, and
            # the two HWDGE rings (sync=SP, scalar=Activation) in parallel —
            # x on the sync ring, all weights on the scalar ring.
            piece_k0 = [0, 6, 10, 13, 16]
            xt0_p = []
            wq_p = []
            wk_h = []
            for e in range(len(piece_k0) - 1):
                k0, k1 = piece_k0[e], piece_k0[e + 1]
                xe = xt0_pool.tile([P, k1 - k0, NSPAN], DT, tag=f"xte{e}", name=f"xt0e{e}")
                nc.sync.dma_start(xe[:], xd[:, 0, k0:k1, :])
                xt0_p.append(xe)
                wt = const.tile([P, (k1 - k0) * 512], DT, tag=f"wqe{e}", name=f"wqe{e}")
                nc.sync.dma_start(wt[:], wqd[:, k0 * 512:k1 * 512])
                wq_p.append(wt)
                t = const.tile([P, (k1 - k0) * 512], DT, tag=f"wkh{e}", name=f"wkh{e}")
                nc.sync.dma_start(t[:], wkd[:, k0 * 512:k1 * 512])
                wk_h.append(t)

            def piece_of(kt):
                for e in range(len(piece_k0) - 1):
                    if piece_k0[e] <= kt < piece_k0[e + 1]:
                        return e, kt - piece_k0[e]
                raise AssertionError
            wv_sb = const.tile([P, KT * 512], DT)
            nc.sync.dma_start(wv_sb[:], wvd[:])
            wo_sb = const.tile([P, H * C], DT)
            nc.sync.dma_start(wo_sb[:], wod[:])
            qdec_sb = const.tile([P, H * NSPAN], F32)
            nc.gpsimd.dma_start(qdec_sb[:], qdecd[:])
            bqk_sb = const.tile([P, 2 * H], F32)
            nc.gpsimd.dma_start(bqk_sb[:], bqkd[:])
            kdecv_sb = const.tile([P, H * HD], F32)
            nc.gpsimd.dma_start(kdecv_sb[:], kdecvd[:])
            bdf_sb = const.tile([P, H * HD], F32)
            nc.gpsimd.dma_start(bdf_sb[:], bdfd[:])
            bvf_sb = const.tile([P, H * HD], F32)
            nc.gpsimd.dma_start(bvf_sb[:], bvfd[:])
            maskt_sb = const.tile([P, H * L], F32)
            nc.gpsimd.dma_start(maskt_sb[:], masktd[:])
            ident = const.tile([P, P], DT)
            from concourse.masks import make_identity
            make_identity(nc, ident)

            # per-head decay state S [d, e], 4 heads side by side, fp32
            S_sb = const.tile([P, H * HD], F32)
            nc.vector.memset(S_sb[:], 0.0)
            S_bf = const.tile([P, H * HD], DT)
            nc.vector.memset(S_bf[:], 0.0)

            xt_tiles = [None]
            for it in range(NIT):
                n0 = it * NSPAN
                # prefetch next span's xT (host-packed, one contiguous tile)
                if it + 1 < NIT:
                    xs = xt_pool.tile([P, KT, NSPAN], DT, tag="xsp", name=f"xsp{it + 1}")
                    nc.sync.dma_start(xs[:], xd[:, it + 1, :, :])
                    xt_tiles.append(xs)

                def wq_sl(kt, h):
                    e, o = piece_of(kt)
                    return wq_p[e][:, o * 512 + h * HD: o * 512 + (h + 1) * HD]

                if it == 0:
                    def xts(kt):
                        e, o = piece_of(kt)
                        return xt0_p[e][:, o, :]
                else:
                    xtab = xt_tiles[it]

                    def xts(kt, xtab=xtab):
                        return xtab[:, kt, :]

                # ---- qkv projection for the span ----
                q_raw = qk_pool.tile([P, H * NSPAN], DT, tag="q_raw")
                q_dec = qk_pool.tile([P, H * NSPAN], DT, tag="q_dec")
                k_sb = qk_pool.tile([P, H * NSPAN], DT, tag="k_sb")
                v_sb = qk_pool.tile([P, H * NSPAN], DT, tag="v_sb")
                vdec = qk_pool.tile([P, H * NSPAN], DT, tag="vdec")

                def wk_sl(kt, h):
                    e, o = piece_of(kt)
                    return wk_h[e][:, o * 512 + h * HD: o * 512 + (h + 1) * HD]

                def evict_q(ps, h):
                    # q_raw = psum + bq (scalar engine); q_dec = (psum + bq) *
                    # qdec (vector engine) — independent reads of the PSUM
                    # bank so the two evictions run in parallel.
                    nc.scalar.activation(
                        q_raw[:, h * NSPAN:(h + 1) * NSPAN], ps[:],
                        mybir.ActivationFunctionType.Identity, bias=bqk_sb[:, 2 * h:2 * h + 1],
                    )
                    nc.vector.scalar_tensor_tensor(
                        q_dec[:, h * NSPAN:(h + 1) * NSPAN], ps[:], bqk_sb[:, 2 * h:2 * h + 1],
                        qdec_sb[:, h * NSPAN:(h + 1) * NSPAN], op0=add, op1=mult,
                    )

                def evict_k(ps, h):
                    nc.scalar.activation(
                        k_sb[:, h * NSPAN:(h + 1) * NSPAN], ps[:],
                        mybir.ActivationFunctionType.Identity, bias=bqk_sb[:, 2 * h + 1:2 * h + 2],
                    )

                if it == 0:
                    # Startup is DMA-gated: run Q AND K kt-striped in 8
                    # parallel PSUM banks (Q in the qkv pool, K borrowing the
                    # idle attention/output banks) so every arriving
                    # x/wq/wk quarter-stripe unlocks 32 matmuls — the PE
                    # stays saturated (and HAM stays warm) through the ramp.
                    psq = [qkv_ps.tile([P, NSPAN], F32, tag="qkvps", name=f"q0ps{h}") for h in range(H)]
                    psk = [
                        attn_ps.tile([P, NSPAN], F32, tag="sc", name="k0ps0"),
                        attn_ps.tile([P, NSPAN], F32, tag="o", name="k0ps1"),
                        out_ps.tile([P, NSPAN], F32, tag="outps", name="k0ps2"),
                        out_ps.tile([P, NSPAN], F32, tag="outps", name="k0ps3"),
                    ]
                    for e in range(len(piece_k0) - 1):
                        k0, k1 = piece_k0[e], piece_k0[e + 1]
                        for kt in range(k0, k1):
                            for h in range(H):
                                nc.tensor.matmul(
                                    psq[h][:], lhsT=wq_sl(kt, h), rhs=xts(kt),
                                    start=(kt == 0), stop=(kt == KT - 1),
                                )
                        for kt in range(k0, k1):
                            for h in range(H):
                                nc.tensor.matmul(
                                    psk[h][:], lhsT=wk_sl(kt, h), rhs=xts(kt),
                                    start=(kt == 0), stop=(kt == KT - 1),
                                )

                    for h in range(H):
                        evict_q(psq[h], h)
                    for h in range(H):
                        evict_k(psk[h], h)
                else:
                    for h in range(H):
                        ps = qkv_ps.tile([P, NSPAN], F32, tag="qkvps")
                        for kt in range(KT):
                            nc.tensor.matmul(
                                ps[:], lhsT=wq_sl(kt, h), rhs=xts(kt),
                                start=(kt == 0), stop=(kt == KT - 1),
                            )
                        evict_q(ps, h)
                        ps = qkv_ps.tile([P, NSPAN], F32, tag="qkvps")
                        for kt in range(KT):
                            nc.tensor.matmul(
                                ps[:], lhsT=wk_sl(kt, h), rhs=xts(kt),
                                start=(kt == 0), stop=(kt == KT - 1),
                            )
                        evict_k(ps, h)

                for ns in range(4):
                    ps = qkv_ps.tile([P, NSPAN], F32, tag="qkvps")
                    for kt in range(KT):
                        nc.tensor.matmul(
                            ps[:],
                            lhsT=xts(kt)[:, ns * P:(ns + 1) * P],
                            rhs=wv_sb[:, kt * 512:(kt + 1) * 512],
                            start=(kt == 0), stop=(kt == KT - 1),
                        )
                    nc.vector.tensor_tensor(v_sb[:, ns * 512:(ns + 1) * 512], ps[:], bvf_sb[:], op=add)
                    nc.vector.tensor_tensor(vdec[:, ns * 512:(ns + 1) * 512], v_sb[:, ns * 512:(ns + 1) * 512], kdecv_sb[:], op=mult)


                # ---- attention + output projection, chunk by chunk ----
                # In the last span the qkv PSUM banks are idle; borrow them so
                # the attention chunks can overlap instead of serializing on
                # the single attn bank pair.
                last = it == NIT - 1
                for p in range(4):
                    ap_pool = qkv_ps if last else attn_ps
                    ap_tag = "qkvps" if last else "sc"
                    ao_tag = "qkvps" if last else "o"
                    # scoresT for all 4 heads into one psum bank
                    sc_ps = ap_pool.tile([P, 512], F32, tag=ap_tag, name=f"sc_ps{p}")
                    for h in range(H):
                        nc.tensor.matmul(
                            sc_ps[:, h * L:(h + 1) * L],
                            lhsT=k_sb[:, h * NSPAN + p * L: h * NSPAN + (p + 1) * L],
                            rhs=q_raw[:, h * NSPAN + p * L: h * NSPAN + (p + 1) * L],
                            start=True, stop=True,
                        )
                    scT = sc_pool.tile([P, 512], DT, tag="scT")
                    nc.vector.tensor_tensor(scT[:], sc_ps[:], maskt_sb[:], op=mult)

                    # After the very last chunk the decay state S is dead, so
                    # all S-maintenance (kT transpose, su matmuls, updates)
                    # is skipped — it would otherwise delay the final
                    # out-projection in the PE FIFO right at the tail.
                    s_live = not (last and p == 3)

                    if s_live:
                        # decay the state early (S_sb is not read by the PE;
                        # only its bf16 shadow is) so the post-su update is
                        # 2 ops.
                        nc.vector.tensor_tensor(S_sb[:], S_sb[:], bdf_sb[:], op=mult)

                        # kT (transpose k chunk) for all 4 heads
                        kt_ps = ap_pool.tile([P, 512], DT, tag=ap_tag, name=f"kt_ps{p}")
                        for h in range(H):
                            nc.tensor.transpose(
                                kt_ps[:, h * HD:(h + 1) * HD],
                                k_sb[:, h * NSPAN + p * L: h * NSPAN + (p + 1) * L],
                                ident[:],
                            )
                        kT = sc_pool.tile([P, 512], DT, tag="kT")
                        nc.scalar.copy(kT[:], kt_ps[:])

                    # o = v^T @ scoresT + S^T @ qdec   [e, i] per head
                    o_ps = ap_pool.tile([P, 512], F32, tag=ao_tag, name=f"o_ps{p}")
                    for h in range(H):
                        nc.tensor.matmul(
                            o_ps[:, h * L:(h + 1) * L],
                            lhsT=v_sb[:, p * 512 + h * HD: p * 512 + (h + 1) * HD],
                            rhs=scT[:, h * L:(h + 1) * L],
                            start=True, stop=False,
                        )
                        nc.tensor.matmul(
                            o_ps[:, h * L:(h + 1) * L],
                            lhsT=S_bf[:, h * HD:(h + 1) * HD],
                            rhs=q_dec[:, h * NSPAN + p * L: h * NSPAN + (p + 1) * L],
                            start=False, stop=True,
                        )
                    ob = ob_pool.tile([P, 512], DT, tag="ob")
                    nc.vector.tensor_copy(ob[:], o_ps[:])

                    if s_live:
                        # S <- S (pre-decayed above) + kT^T @ vdec
                        su_ps = ap_pool.tile([P, 512], F32, tag=ao_tag, name=f"su_ps{p}")
                        for h in range(H):
                            nc.tensor.matmul(
                                su_ps[:, h * HD:(h + 1) * HD],
                                lhsT=kT[:, h * HD:(h + 1) * HD],
                                rhs=vdec[:, p * 512 + h * HD: p * 512 + (h + 1) * HD],
                                start=True, stop=True,
                            )
                        # bf16 shadow first — the next chunk's o-inter matmul
                        # only needs S_bf; the f32 state update follows off
                        # the critical path.
                        nc.vector.tensor_tensor(S_bf[:], S_sb[:], su_ps[:], op=add)
                        nc.vector.tensor_tensor(S_sb[:], S_sb[:], su_ps[:], op=add)

                    # output projection for this chunk's 128 tokens (bf16 partials)
                    outb = outb_pool.tile([P, C], DT, tag="outb")
                    for ct in range(4):
                        ops = out_ps.tile([P, 512], F32, tag="outps")
                        for h in range(H):
                            nc.tensor.matmul(
                                ops[:],
                                lhsT=ob[:, h * L:(h + 1) * L],
                                rhs=wo_sb[:, h * C + ct * 512: h * C + (ct + 1) * 512],
                                start=(h == 0), stop=(h == H - 1),
                            )
                        if last and p == 3 and ct == 3:
                            # Final piece: evict in halves on both engines
                            # and flush each half on its own HWDGE ring so
                            # the kernel tail is as short as possible.
                            c0 = ct * 512
                            nc.vector.tensor_copy(outb[:, c0:c0 + 256], ops[:, 0:256])
                            nc.scalar.copy(outb[:, c0 + 256:c0 + 512], ops[:, 256:512])
                            nc.sync.dma_start(
                                outd[n0 + p * L: n0 + (p + 1) * L, c0:c0 + 256],
                                outb[:, c0:c0 + 256],
                            )
                            nc.scalar.dma_start(
                                outd[n0 + p * L: n0 + (p + 1) * L, c0 + 256:c0 + 512],
                                outb[:, c0 + 256:c0 + 512],
                            )
                            continue
                        if ct % 2 == 0:
                            nc.vector.tensor_copy(outb[:, ct * 512:(ct + 1) * 512], ops[:])
                        else:
                            nc.scalar.copy(outb[:, ct * 512:(ct + 1) * 512], ops[:])
                        # The very last chunk streams out piece-wise on
                        # alternating HWDGE rings so the final flush overlaps
                        # the out-projection instead of trailing it.
                        if last and p == 3:
                            ring = nc.sync if ct % 2 == 0 else nc.scalar
                            ring.dma_start(
                                outd[n0 + p * L: n0 + (p + 1) * L, ct * 512:(ct + 1) * 512],
                                outb[:, ct * 512:(ct + 1) * 512],
                            )
                    # outputs ride the fast HWDGE sync ring (idle mid-run);
                    # the slow SWDGE ring only carries the small const loads.
                    if last and p == 3:
                        pass
                    elif last and p == 2:
                        nc.scalar.dma_start(outd[n0 + p * L: n0 + (p + 1) * L, :], outb[:])
                    else:
                        nc.sync.dma_start(outd[n0 + p * L: n0 + (p + 1) * L, :], outb[:])

    nc.compile()
    return nc


def _host_inputs(x, Wqkv, bqkv, Wout, bout, slopes):
    """Per-core input maps (numpy, host-side sharding + packing)."""
    in_maps = []
    # packed transpose of x, shared by the 4 cores of each batch:
    # xtp[c, it, kt, n'] = x[b, it*512 + n', kt*128 + c]
    _xtp_cache = [
        np.ascontiguousarray(
            x[b].astype(BF16).reshape(NIT, NSPAN, KT, P).transpose(3, 0, 2, 1)
        )
        for b in range(B)
    ]
    i = np.arange(L, dtype=np.float64)
    for core in range(8):
        b, g = core // 4, core % 4
        h0 = 4 * g
        hsel = slice(h0 * HD, (h0 + H) * HD)

        xb = _xtp_cache[b]

        def pack_w(Wslice):
            # (C, 512) -> [c_in_tile(128), kt*512 + col]
            return np.ascontiguousarray(
                Wslice.astype(BF16).reshape(KT, P, H * HD).transpose(1, 0, 2).reshape(P, KT * 512)
            )

        wq = pack_w(Wqkv[:, 0 * C:1 * C][:, hsel])
        wk = pack_w(Wqkv[:, 1 * C:2 * C][:, hsel])
        wv = pack_w(Wqkv[:, 2 * C:3 * C][:, hsel])
        # Wout rows for these heads: [d(128), h*2048 + outc]
        wo = np.ascontiguousarray(
            Wout[hsel, :].astype(BF16).reshape(H, HD, C).transpose(1, 0, 2).reshape(P, H * C)
        )

        s = slopes[h0:h0 + H].astype(np.float64)  # (4,)
        diffT = (i[None, :] - i[:, None])          # [j, i] = i - j
        maskt = np.concatenate(
            [np.where(diffT >= 0, np.exp(-s[h] * diffT), 0.0) for h in range(H)],
            axis=1,
        ).astype(np.float32)                       # [128, 4*128]
        qdec_l = [np.exp(-s[h] * i) for h in range(H)]        # each (L,)
        qdec = np.concatenate(
            [np.broadcast_to(np.tile(qdec_l[h], NSPAN // L)[None, :], (P, NSPAN)) for h in range(H)],
            axis=1,
        ).astype(np.float32)                       # [128, 4*512]
        kdecv = np.concatenate(
            [np.broadcast_to(np.exp(-s[h] * (L - i))[:, None], (P, HD)) for h in range(H)],
            axis=1,
        ).astype(np.float32)                       # [128, 4*128]
        bdf = np.concatenate(
            [np.full((P, HD), math.exp(-s[h] * L)) for h in range(H)], axis=1
        ).astype(np.float32)
        # per-head, per-partition(d) q/k biases: columns [bq_h0, bk_h0, bq_h1, ...]
        bq_heads = bqkv[0 * C:1 * C][hsel].reshape(H, HD)
        bk_heads = bqkv[1 * C:2 * C][hsel].reshape(H, HD)
        bqk = np.zeros((P, 2 * H), dtype=np.float32)
        for h in range(H):
            bqk[:, 2 * h] = bq_heads[h]
            bqk[:, 2 * h + 1] = bk_heads[h]
        bvf = np.broadcast_to(bqkv[2 * C:3 * C][hsel][None, :], (P, H * HD)).astype(np.float32)

        in_maps.append({
            "x": xb, "wq": wq, "wk": wk, "wv": wv, "wo": wo,
            "maskt": maskt, "qdec": qdec, "kdecv": kdecv, "bdf": bdf,
            "bqk": bqk, "bvf": np.ascontiguousarray(bvf),
        })
    return in_maps


def kernel(x, Wqkv, bqkv, Wout, bout, slopes, _want_trace=False):
    from concourse import bass_utils

    x = np.asarray(x, dtype=np.float32)
    Wqkv = np.asarray(Wqkv, dtype=np.float32)
    bqkv = np.asarray(bqkv, dtype=np.float32)
    Wout = np.asarray(Wout, dtype=np.float32)
    bout = np.asarray(bout, dtype=np.float32)
    slopes = np.asarray(slopes, dtype=np.float32)

    if "nc" not in _CACHE:
        _CACHE["nc"] = _build()
    nc = _CACHE["nc"]

    in_maps = _host_inputs(x, Wqkv, bqkv, Wout, bout, slopes)
    res = bass_utils.run_bass_kernel_spmd(
        nc, in_maps, core_ids=list(range(8)), trace=_want_trace,
    )
    out = np.zeros((B, N, C), dtype=np.float32)
    for core in range(8):
        out[core // 4] += res.results[core]["out"].astype(np.float32)
    out += bout[None, None, :]
    if _want_trace:
        _CACHE["last_result"] = res
    return out
